# revision 1
# baseline (speedup 1.0000x reference)
"""3-layer GCN on 8 Trainium2 NeuronCores (Bass/Tile SPMD kernel).

Strategy (sharding_hint: shard nodes + edges by destination, replicate
weights, all-gather activations between layers):

  - Nodes are split into 8 contiguous blocks (padded to a multiple of 128
    rows per core).  Core c owns destination block c.
  - Per layer, using linearity of GCN aggregation:
        out_i = [sum_{j->i} dinv_i dinv_j p_j + dinv_i^2 p_i] @ W + b
    with p = previous activations.  We store ps = dinv * p ("scaled"
    activations) so every message (including the self loop, added as an
    explicit edge) has unit coefficient:
        z_i = dinv_i * segment_sum(ps[src])       (over edges + self edges)
        out_i = z_i @ W + b ; next ps = dinv * relu(out)
  - Each core DMA-gathers ps[src] rows (512B each) for its edges from a
    replicated full-activation DRAM buffer, reduces them into per-256-dst
    "window pair" PSUM tiles with one-hot matmuls (one-hots built by DVE
    is_equal against an iota row), applies dinv, multiplies by W (PE),
    bias+ReLU (ACT), rescales, and writes its 1/8 output block.
  - An AllGather (ncfw collective) replicates the per-core ps blocks
    between layers.  Matmul operands use float32r (TF32-like, ~1e-4 rel).

Edges are sorted by (dst core, dst window pair, src chunk); each
(pair, chunk) cell is padded to whole 128-token groups with a group count
equalized across cores so a single SPMD program serves all 8 cores.
Gather indices are int16 (chunk-relative, chunks of <=32768 rows).
"""

import sys

if "/opt/trn_rl_repo" not in sys.path:
    sys.path.insert(0, "/opt/trn_rl_repo")

import numpy as np

import concourse.bacc as bacc
import concourse.mybir as mybir
import concourse.tile as tile
from concourse import bass_utils

F32 = mybir.dt.float32
F32R = mybir.dt.float32r
F16 = mybir.dt.float16
I16 = mybir.dt.int16

NCORES = 8
D = 128
USE_F32R = True      # float32r (TF32-like) matmul operands; False = exact fp32
USE_F16_MSG = True   # fp16 gathered messages + one-hot S (halves gather bytes)
NQ = 4               # SWDGE queues for gather desc-gen parallelism
PAIRW = 256          # dst window-pair width (S matrix / PSUM free size)
CHUNK_ROWS = 25088   # gather source chunk rows (<= 32768 for int16 idx)
RP = 2               # window pairs per gather range


def _preprocess(edge_index, n_nodes):
    """Host-side integer preprocessing: degrees, edge partition, padding.

    Returns a dict with the static program structure (identical across
    cores) and per-core gather/one-hot metadata arrays.
    """
    src = np.asarray(edge_index[0], dtype=np.int64)
    dst = np.asarray(edge_index[1], dtype=np.int64)

    nb_real = -(-n_nodes // NCORES)              # real rows per core
    NB = -(-nb_real // 128) * 128                # padded rows per core
    NPAD = NB * NCORES
    NW = NB // 128                               # windows per core
    NPAIR = -(-NW // 2)                          # window pairs per core
    nchunks = -(-NPAD // CHUNK_ROWS)

    deg = np.ones(n_nodes, dtype=np.float64)
    np.add.at(deg, dst, 1.0)                     # bincount, +1 self loop
    deg = deg.astype(np.float32)

    # global padded row id of each node
    def gp(n):
        return (n // nb_real) * NB + (n % nb_real)

    # append self edges
    allnodes = np.arange(n_nodes, dtype=np.int64)
    s_all = np.concatenate([src, allnodes])
    d_all = np.concatenate([dst, allnodes])

    core = d_all // nb_real
    dloc = d_all % nb_real
    pair = dloc // PAIRW
    poff = dloc - pair * PAIRW                   # offset within pair [0, 256)
    sgp = gp(s_all)
    chunk = sgp // CHUNK_ROWS
    sidx = (sgp - chunk * CHUNK_ROWS).astype(np.int64)

    # sort by (core, pair, chunk), then src within each cell (HBM locality)
    key = ((core * NPAIR) + pair) * nchunks + chunk
    order = np.lexsort((sidx, key))
    key_s = key[order]
    sidx_s = sidx[order]
    poff_s = poff[order]

    ncells = NPAIR * nchunks
    # per-core per-cell counts
    counts = np.zeros((NCORES, ncells), dtype=np.int64)
    uk, uc = np.unique(key_s, return_counts=True)
    counts.reshape(-1)[uk] = uc
    gcell = (-(-counts // 128)).max(axis=0)      # equalized group counts
    gcell = gcell.reshape(NPAIR, nchunks)        # [pair, chunk]

    gtot = int(gcell.sum())
    # stream layout: for p in pairs: for k in chunks: gcell[p,k] groups
    cell_goff = np.zeros((NPAIR, nchunks), dtype=np.int64)
    g = 0
    for p in range(NPAIR):
        for k in range(nchunks):
            cell_goff[p, k] = g
            g += gcell[p, k]

    # per-core padded token arrays in stream order (token-major flat)
    dstw_flat = np.full((NCORES, gtot * 128), -1.0, dtype=np.float32)

    # chunk stream group offsets (within each chunk's gather stream)
    chunk_goff = np.zeros((NPAIR, nchunks), dtype=np.int64)
    acc = np.zeros(nchunks, dtype=np.int64)
    for p in range(NPAIR):
        for k in range(nchunks):
            chunk_goff[p, k] = acc[k]
            acc[k] += gcell[p, k]
    gchunk = acc                                  # groups per chunk stream

    idx_streams = [
        np.zeros((NCORES, int(gchunk[k]) * 128), dtype=np.int16)
        for k in range(nchunks)
    ]

    cell_starts = np.zeros(NCORES * ncells + 1, dtype=np.int64)
    np.cumsum(counts.reshape(-1), out=cell_starts[1:])
    for c in range(NCORES):
        for p in range(NPAIR):
            for k in range(nchunks):
                cell = (c * NPAIR + p) * nchunks + k
                t0, t1 = cell_starts[cell], cell_starts[cell + 1]
                n = t1 - t0
                gk0 = chunk_goff[p, k] * 128
                idx_streams[k][c, gk0 : gk0 + n] = sidx_s[t0:t1]
                g0 = cell_goff[p, k]
                dstw_flat[c, g0 * 128 : g0 * 128 + n] = poff_s[t0:t1]
                # pads keep idx 0 / dstw -1

    # dstw: token t of group g -> [t%128, g]
    dstw = np.ascontiguousarray(
        dstw_flat.reshape(NCORES, gtot, 128).transpose(0, 2, 1)
    )

    # wrap indices: token i -> [i%16, i//16], replicated to 128 partitions
    idx_wrapped = []
    for k in range(nchunks):
        st = idx_streams[k]
        cols = st.shape[1] // 16
        w = st.reshape(NCORES, cols, 16).transpose(0, 2, 1)  # [C,16,cols]
        idx_wrapped.append(np.tile(w, (1, 8, 1)).copy())     # [C,128,cols]

    # degree layouts
    deg_pad = np.ones(NPAD, dtype=np.float32)
    for c in range(NCORES):
        lo = c * nb_real
        hi = min(n_nodes, (c + 1) * nb_real)
        deg_pad[c * NB : c * NB + (hi - lo)] = deg[lo:hi]
    deg_w = np.empty((NCORES, 128, NW), dtype=np.float32)    # wrapped
    deg_r = np.empty((NCORES, 1, NB), dtype=np.float32)      # row
    for c in range(NCORES):
        blk = deg_pad[c * NB : (c + 1) * NB]
        deg_w[c] = blk.reshape(NW, 128).T
        deg_r[c, 0] = blk

    return dict(
        NB=NB, NPAD=NPAD, NW=NW, NPAIR=NPAIR, nchunks=nchunks,
        nb_real=nb_real, gcell=gcell, gtot=gtot, gchunk=gchunk,
        cell_goff=cell_goff, chunk_goff=chunk_goff,
        idx_wrapped=idx_wrapped, dstw=dstw, deg_w=deg_w, deg_r=deg_r,
    )


def _build(meta):
    NB, NPAD, NW, NPAIR = meta["NB"], meta["NPAD"], meta["NW"], meta["NPAIR"]
    nchunks, gcell, gtot = meta["nchunks"], meta["gcell"], meta["gtot"]
    gchunk, cell_goff, chunk_goff = (
        meta["gchunk"], meta["cell_goff"], meta["chunk_goff"],
    )

    DT_R = F32R if USE_F32R else F32
    DT_M = F16 if USE_F16_MSG else DT_R     # message/S dtype

    nc = bacc.Bacc(None, target_bir_lowering=False, num_devices=NCORES,
                   num_swdge_queues=NQ)

    x_ext = nc.dram_tensor("x", [NB, D], F32, kind="ExternalInput")
    degw_ext = nc.dram_tensor("degw", [128, NW], F32, kind="ExternalInput")
    iota_ext = nc.dram_tensor("iota", [128, PAIRW],
                              F16 if USE_F16_MSG else F32,
                              kind="ExternalInput")
    ident_ext = nc.dram_tensor("ident", [128, 128], F32, kind="ExternalInput")
    ones_ext = nc.dram_tensor("ones1", [1, 128], F32, kind="ExternalInput")
    w_ext = [
        nc.dram_tensor(f"w{l}", [D, D], F32, kind="ExternalInput")
        for l in range(3)
    ]
    b_ext = [
        nc.dram_tensor(f"b{l}", [128, 1], F32, kind="ExternalInput")
        for l in range(3)
    ]
    idx_ext = [
        nc.dram_tensor(f"idx{k}", [128, int(gchunk[k]) * 8], I16,
                       kind="ExternalInput")
        for k in range(nchunks)
    ]
    dstw_ext = nc.dram_tensor("dstw", [128, gtot], F32, kind="ExternalInput")
    out_ext = nc.dram_tensor("out", [NB, D], F32, kind="ExternalOutput")

    ps_loc = nc.dram_tensor("ps_loc", [NB, D], DT_M)
    ps_full = nc.dram_tensor("ps_full", [NPAD, D], DT_M, addr_space="Shared")

    QROT = [0]

    # gather ranges: RP pairs each
    ranges = [list(range(r, min(r + RP, NPAIR))) for r in range(0, NPAIR, RP)]

    with tile.TileContext(nc) as tc:
        with (
            tc.tile_pool(name="const", bufs=1) as cpool,
            tc.tile_pool(name="msg", bufs=3) as mpool,
            tc.tile_pool(name="idxp", bufs=2) as ipool,
            tc.tile_pool(name="sbld", bufs=8) as spool,
            tc.tile_pool(name="work", bufs=3) as wpool,
            tc.tile_pool(name="outp", bufs=4) as opool,
            tc.tile_pool(name="pz", bufs=2, space="PSUM") as pzpool,
            tc.tile_pool(name="pt", bufs=2, space="PSUM") as ptpool,
            tc.tile_pool(name="ph", bufs=2, space="PSUM") as phpool,
        ):
            # ---- constants ----
            iota_sb = cpool.tile([128, PAIRW], F16 if USE_F16_MSG else F32)
            nc.sync.dma_start(out=iota_sb[:], in_=iota_ext[:, :])
            ident_sb = cpool.tile([128, 128], F32)
            nc.sync.dma_start(out=ident_sb[:], in_=ident_ext[:, :])
            ones_sb = cpool.tile([1, 128], F32)
            nc.sync.dma_start(out=ones_sb[:], in_=ones_ext[:, :])
            w_sb = []
            for l in range(3):
                wt = cpool.tile([D, D], F32, tag=f"wraw{l}")
                nc.sync.dma_start(out=wt[:], in_=w_ext[l][:, :])
                if USE_F32R:
                    wr = cpool.tile([D, D], F32R, tag=f"w{l}")
                    nc.vector.tensor_copy(wr[:], wt[:])
                    w_sb.append(wr)
                else:
                    w_sb.append(wt)
            b_sb = []
            for l in range(3):
                bt = cpool.tile([128, 1], F32, tag=f"b{l}")
                nc.sync.dma_start(out=bt[:], in_=b_ext[l][:, :])
                b_sb.append(bt)
            dstw_sb = cpool.tile([128, gtot], F32)
            nc.sync.dma_start(out=dstw_sb[:], in_=dstw_ext[:, :])

            # ---- dinv (wrapped + broadcast along free dim) ----
            degw_sb = cpool.tile([128, NW], F32)
            nc.sync.dma_start(out=degw_sb[:], in_=degw_ext[:, :])
            rcpw = cpool.tile([128, NW], F32)
            nc.vector.reciprocal(rcpw[:], degw_sb[:])
            dinv_w = cpool.tile([128, NW], F32)
            nc.scalar.activation(dinv_w[:], rcpw[:],
                                 mybir.ActivationFunctionType.Sqrt)

            # dinv_bc[p, d] = dinv[d]: per window, move the dinv column to a
            # partition-0 row (matmul vs identity), then K=1 ones-broadcast.
            dinv_bc = cpool.tile([128, NB], F32)
            for w in range(NW):
                pr = ptpool.tile([128, 128], F32, tag="tp")
                nc.tensor.matmul(pr[0:1, :], dinv_w[:, w : w + 1],
                                 ident_sb[:], start=True, stop=True)
                row_sb = wpool.tile([1, 128], F32, tag="drow")
                nc.scalar.copy(out=row_sb[:], in_=pr[0:1, :])
                pb = ptpool.tile([128, 128], F32, tag="tp")
                nc.tensor.matmul(pb[:], ones_sb[:], row_sb[:],
                                 start=True, stop=True)
                nc.scalar.copy(
                    out=dinv_bc[:, w * 128 : w * 128 + 128], in_=pb[:]
                )

            # ---- prologue: ps0 = dinv * x ----
            for w in range(NW):
                xt = wpool.tile([128, 128], F32, tag="xin")
                nc.sync.dma_start(out=xt[:], in_=x_ext[w * 128 : w * 128 + 128, :])
                xs = opool.tile([128, 128], DT_M, tag="psout")
                nc.vector.tensor_scalar(
                    xs[:], xt[:], dinv_w[:, w : w + 1], None,
                    op0=mybir.AluOpType.mult,
                )
                nc.sync.dma_start(
                    out=ps_loc[w * 128 : w * 128 + 128, :], in_=xs[:]
                )
            nc.gpsimd.collective_compute(
                "AllGather", mybir.AluOpType.bypass,
                replica_groups=[list(range(NCORES))],
                ins=[ps_loc.ap().opt()], outs=[ps_full.ap().opt()],
            )

            # ---- layers ----
            for layer in range(3):
                for rng_pairs in ranges:
                    # gather all chunks for this range
                    mtiles = {}
                    for k in range(nchunks):
                        g_rk = int(sum(gcell[p, k] for p in rng_pairs))
                        if g_rk == 0:
                            continue
                        g0 = int(chunk_goff[rng_pairs[0], k])
                        ni = g_rk * 128
                        it = ipool.tile([128, ni // 16], I16, tag=f"i{k}")
                        nc.sync.dma_start(
                            out=it[:],
                            in_=idx_ext[k][:, g0 * 8 : g0 * 8 + ni // 16],
                        )
                        mt = mpool.tile([128, g_rk, 128], DT_M, tag=f"m{k}")
                        nc.gpsimd.dma_gather(
                            mt[:],
                            ps_full[k * CHUNK_ROWS : (k + 1) * CHUNK_ROWS, :],
                            it[:],
                            ni, ni, D,
                            single_packet=False,
                            queue_num=QROT[0] % NQ,
                        )
                        QROT[0] += 1
                        mtiles[k] = (mt, g0)

                    for p in rng_pairs:
                        # segment-sum into PSUM [feat, PAIRW]
                        zps = pzpool.tile([128, PAIRW], F32, tag="zacc")
                        ng = int(sum(gcell[p, k] for k in range(nchunks)))
                        gi = 0
                        for k in range(nchunks):
                            for j in range(int(gcell[p, k])):
                                mt, g0 = mtiles[k]
                                slot = int(chunk_goff[p, k]) - g0 + j
                                gcol = int(cell_goff[p, k]) + j
                                s_t = spool.tile([128, PAIRW], DT_M, tag="s")
                                nc.vector.tensor_scalar(
                                    s_t[:], iota_sb[:],
                                    dstw_sb[:, gcol : gcol + 1], None,
                                    op0=mybir.AluOpType.is_equal,
                                )
                                nc.tensor.matmul(
                                    zps[:], mt[:, slot, :], s_t[:],
                                    start=(gi == 0), stop=(gi == ng - 1),
                                )
                                gi += 1

                        # z^T = dinv ⊙ u^T ; -> SBUF f32r (rhs of W matmul)
                        zsT = wpool.tile([128, PAIRW], DT_R, tag="zst")
                        c0 = p * PAIRW
                        nc.vector.tensor_mul(
                            zsT[:], zps[:], dinv_bc[:, c0 : c0 + PAIRW]
                        )

                        hps = phpool.tile([128, PAIRW], F32, tag="h")
                        nc.tensor.matmul(
                            hps[:], w_sb[layer][:], zsT[:],
                            start=True, stop=True,
                        )
                        hT = wpool.tile([128, PAIRW], F32, tag="ht")
                        if layer < 2:
                            nc.scalar.activation(
                                hT[:], hps[:],
                                mybir.ActivationFunctionType.Relu,
                                bias=b_sb[layer][:],
                            )
                        else:
                            nc.scalar.activation(
                                hT[:], hps[:],
                                mybir.ActivationFunctionType.Identity,
                                bias=b_sb[layer][:],
                            )
                        for h in range(2):
                            w = p * 2 + h
                            if w >= NW:
                                break
                            tp = ptpool.tile([128, 128], F32, tag="tp")
                            nc.tensor.transpose(
                                tp[:], hT[:, h * 128 : h * 128 + 128],
                                ident_sb[:],
                            )
                            if layer < 2:
                                pst = opool.tile([128, 128], DT_M, tag="psout")
                                nc.vector.tensor_scalar(
                                    pst[:], tp[:], dinv_w[:, w : w + 1], None,
                                    op0=mybir.AluOpType.mult,
                                )
                                nc.sync.dma_start(
                                    out=ps_loc[w * 128 : w * 128 + 128, :],
                                    in_=pst[:],
                                )
                            else:
                                ot = opool.tile([128, 128], F32, tag="oout")
                                nc.scalar.copy(out=ot[:], in_=tp[:])
                                nc.sync.dma_start(
                                    out=out_ext[w * 128 : w * 128 + 128, :],
                                    in_=ot[:],
                                )
                if layer < 2:
                    nc.gpsimd.collective_compute(
                        "AllGather", mybir.AluOpType.bypass,
                        replica_groups=[list(range(NCORES))],
                        ins=[ps_loc.ap().opt()], outs=[ps_full.ap().opt()],
                    )

    nc.finalize()
    return nc


_CACHE = {}
TRACE = False          # set by test harness to profile + fill LAST_EXEC_NS
LAST_EXEC_NS = None


def kernel(x, edge_index, W1, b1, W2, b2, W3, b3):
    global LAST_EXEC_NS
    x = np.asarray(x, dtype=np.float32)
    edge_index = np.asarray(edge_index)
    n_nodes = x.shape[0]

    ck = (n_nodes, edge_index.shape[1],
          hash(edge_index.tobytes()))
    if ck in _CACHE:
        meta, nc = _CACHE[ck]
    else:
        meta = _preprocess(edge_index, n_nodes)
        nc = _build(meta)
        _CACHE[ck] = (meta, nc)

    NB, NW, nb_real = meta["NB"], meta["NW"], meta["nb_real"]
    nchunks = meta["nchunks"]

    iota_dt = np.float16 if USE_F16_MSG else np.float32
    iota = np.tile(np.arange(PAIRW, dtype=iota_dt), (128, 1))
    ident = np.eye(128, dtype=np.float32)
    ones1 = np.ones((1, 128), dtype=np.float32)
    ws = [np.asarray(W1, np.float32), np.asarray(W2, np.float32),
          np.asarray(W3, np.float32)]
    bs = [np.asarray(b1, np.float32), np.asarray(b2, np.float32),
          np.asarray(b3, np.float32)]

    in_maps = []
    for c in range(NCORES):
        lo = c * nb_real
        hi = min(n_nodes, (c + 1) * nb_real)
        xb = np.zeros((NB, D), dtype=np.float32)
        xb[: hi - lo] = x[lo:hi]
        im = {
            "x": xb,
            "degw": meta["deg_w"][c],
            "iota": iota,
            "ident": ident,
            "ones1": ones1,
            "dstw": meta["dstw"][c],
        }
        for l in range(3):
            im[f"w{l}"] = ws[l]
            im[f"b{l}"] = bs[l].reshape(128, 1)
        for k in range(nchunks):
            im[f"idx{k}"] = meta["idx_wrapped"][k][c]
        in_maps.append(im)

    res = bass_utils.run_bass_kernel_spmd(
        nc, in_maps, core_ids=list(range(NCORES)), trace=TRACE
    )
    LAST_EXEC_NS = res.exec_time_ns

    out = np.empty((n_nodes, D), dtype=np.float32)
    for c in range(NCORES):
        lo = c * nb_real
        hi = min(n_nodes, (c + 1) * nb_real)
        out[lo:hi] = res.results[c]["out"][: hi - lo]
    return out



# revision 3
# speedup vs baseline: 1.1537x; 1.1537x over previous
"""3-layer GCN on 8 Trainium2 NeuronCores (Bass/Tile SPMD kernel), v2.

Differences from v1:
  - The one-hot S matrices (token -> dst-offset, per 128-token group) are
    precomputed on the host and streamed from DRAM instead of being built
    per-group on DVE with is_equal (which was the v1 bottleneck: ~5 ms DVE).
  - Self-loop edges are not gathered; the self term ps_w is added into the
    PSUM accumulator with an identity matmul from a resident SBUF copy of
    the local ps block.
  - dinv (dst side) is applied on DVE (zsT = zps * dinv_bc); src side is
    folded into ps as before (ps = dinv * act).
  - Gathers rotate across 4 SWDGE queues; everything else (W matmul, bias+
    ReLU on ACT, PE transpose, rescale, AllGather between layers) as in v1.

Shapes: 100000 nodes, 1.6M edges, D=128, NB=12544 rows/core (98 windows,
49 pairs of 256 dst), 4 src chunks of 25088 rows (int16 gather indices).
"""

import sys

if "/opt/trn_rl_repo" not in sys.path:
    sys.path.insert(0, "/opt/trn_rl_repo")

import numpy as np

import concourse.bacc as bacc
import concourse.mybir as mybir
import concourse.tile as tile
from concourse import bass_utils

F32 = mybir.dt.float32
F32R = mybir.dt.float32r
F16 = mybir.dt.float16
F8 = mybir.dt.float8e4
I16 = mybir.dt.int16

NCORES = 8
D = 128
NQ = 4               # SWDGE queues
PAIRW = 256          # dst window-pair width (PSUM tile free size)
RP = 2               # pairs per gather range
S_DTYPE = F8         # one-hot S matrix dtype (exact 0/1 in fp8e4m3)

CHUNK_ROWS = 25088   # gather source chunk rows (<= 32768 for int16 idx)


def _preprocess(edge_index, n_nodes):
    """Host-side preprocessing: edge partition, padding, S/idx blobs."""
    src = np.asarray(edge_index[0], dtype=np.int64)
    dst = np.asarray(edge_index[1], dtype=np.int64)

    nb_real = -(-n_nodes // NCORES)
    NB = -(-nb_real // 128) * 128
    NPAD = NB * NCORES
    NW = NB // 128
    NPAIR = -(-NW // 2)
    nchunks = 4

    deg = np.ones(n_nodes, dtype=np.float64)
    np.add.at(deg, dst, 1.0)
    dinv = (1.0 / np.sqrt(deg)).astype(np.float32)

    core = dst // nb_real
    dloc = dst % nb_real
    pair = dloc // PAIRW
    poff = dloc - pair * PAIRW
    sgp = (src // nb_real) * NB + (src % nb_real)
    chunk = sgp // CHUNK_ROWS
    sidx = (sgp - chunk * CHUNK_ROWS).astype(np.int64)

    # sort by (core, pair, chunk, src)
    key = ((core * NPAIR) + pair) * nchunks + chunk
    order = np.lexsort((sidx, key))
    key_s = key[order]
    sidx_s = sidx[order]
    poff_s = poff[order]

    ncells = NPAIR * nchunks
    counts = np.zeros((NCORES, ncells), dtype=np.int64)
    uk, uc = np.unique(key_s, return_counts=True)
    counts.reshape(-1)[uk] = uc
    gcell = (-(-counts // 128)).max(axis=0).reshape(NPAIR, nchunks)

    gtot = int(gcell.sum())

    # group processing order: pair-major, chunk inner
    cell_goff = np.zeros((NPAIR, nchunks), dtype=np.int64)
    g = 0
    for p in range(NPAIR):
        for k in range(nchunks):
            cell_goff[p, k] = g
            g += gcell[p, k]

    # chunk idx stream offsets (gather stream per chunk, range/pair-major)
    chunk_goff = np.zeros((NPAIR, nchunks), dtype=np.int64)
    acc = np.zeros(nchunks, dtype=np.int64)
    for p in range(NPAIR):
        for k in range(nchunks):
            chunk_goff[p, k] = acc[k]
            acc[k] += gcell[p, k]
    gchunk = acc

    idx_streams = [
        np.zeros((NCORES, int(gchunk[k]) * 128), dtype=np.int16)
        for k in range(nchunks)
    ]
    s_dt = np.float16  # host builds in fp16; cast to fp8 via ml_dtypes if set
    s_blob = np.zeros((NCORES, 128, gtot * PAIRW), dtype=s_dt)

    cell_starts = np.zeros(NCORES * ncells + 1, dtype=np.int64)
    np.cumsum(counts.reshape(-1), out=cell_starts[1:])
    for c in range(NCORES):
        for p in range(NPAIR):
            for k in range(nchunks):
                cell = (c * NPAIR + p) * nchunks + k
                t0, t1 = cell_starts[cell], cell_starts[cell + 1]
                n = int(t1 - t0)
                if n == 0:
                    continue
                gk0 = int(chunk_goff[p, k]) * 128
                idx_streams[k][c, gk0 : gk0 + n] = sidx_s[t0:t1]
                # S blob: token t (slot t%128 of group j) -> col
                # (cell_goff+j)*PAIRW + poff
                t_arr = np.arange(n)
                slot = t_arr % 128
                grp = int(cell_goff[p, k]) + t_arr // 128
                s_blob[c, slot, grp * PAIRW + poff_s[t0:t1]] = 1.0

    idx_wrapped = []
    for k in range(nchunks):
        st = idx_streams[k]
        cols = st.shape[1] // 16
        w = st.reshape(NCORES, cols, 16).transpose(0, 2, 1)
        idx_wrapped.append(np.tile(w, (1, 8, 1)).copy())

    # dinv layouts
    dinv_pad = np.zeros(NPAD, dtype=np.float32)
    for c in range(NCORES):
        lo = c * nb_real
        hi = min(n_nodes, (c + 1) * nb_real)
        dinv_pad[c * NB : c * NB + (hi - lo)] = dinv[lo:hi]
    dinv_w = np.empty((NCORES, 128, NW), dtype=np.float32)   # wrapped cols
    dinv_bc = np.empty((NCORES, 1, NB), dtype=np.float32)    # row (for bcast)
    for c in range(NCORES):
        blk = dinv_pad[c * NB : (c + 1) * NB]
        dinv_w[c] = blk.reshape(NW, 128).T
        dinv_bc[c, 0] = blk

    return dict(
        NB=NB, NPAD=NPAD, NW=NW, NPAIR=NPAIR, nchunks=nchunks,
        nb_real=nb_real, gcell=gcell, gtot=gtot, gchunk=gchunk,
        cell_goff=cell_goff, chunk_goff=chunk_goff,
        idx_wrapped=idx_wrapped, s_blob=s_blob,
        dinv_w=dinv_w, dinv_bc=dinv_bc,
    )


def _build(meta):
    NB, NPAD, NW, NPAIR = meta["NB"], meta["NPAD"], meta["NW"], meta["NPAIR"]
    nchunks, gcell, gtot = meta["nchunks"], meta["gcell"], meta["gtot"]
    gchunk, cell_goff, chunk_goff = (
        meta["gchunk"], meta["cell_goff"], meta["chunk_goff"],
    )

    nc = bacc.Bacc(None, target_bir_lowering=False, num_devices=NCORES,
                   num_swdge_queues=NQ)

    x_ext = nc.dram_tensor("x16", [NB, D], F16, kind="ExternalInput")
    dinvw_ext = nc.dram_tensor("dinvw", [128, NW], F32, kind="ExternalInput")
    dinvbc_ext = nc.dram_tensor("dinvbc", [128, NB], F16,
                                kind="ExternalInput")
    ident_ext = nc.dram_tensor("ident", [128, 128], F16, kind="ExternalInput")
    identf_ext = nc.dram_tensor("identf", [128, 128], F32,
                                kind="ExternalInput")
    w_ext = [
        nc.dram_tensor(f"w{l}", [D, D], F16, kind="ExternalInput")
        for l in range(3)
    ]
    b_ext = [
        nc.dram_tensor(f"b{l}", [128, 1], F32, kind="ExternalInput")
        for l in range(3)
    ]
    idx_ext = [
        nc.dram_tensor(f"idx{k}", [128, int(gchunk[k]) * 8], I16,
                       kind="ExternalInput")
        for k in range(nchunks)
    ]
    s_ext = nc.dram_tensor("sblob", [128, gtot * PAIRW], S_DTYPE,
                           kind="ExternalInput")
    out_ext = nc.dram_tensor("out", [NB, D], F32, kind="ExternalOutput")

    ps_loc = nc.dram_tensor("ps_loc", [NB, D], F16)
    # ping-pong full-activation buffers: layer L gathers from ps_full[L%2],
    # its outputs AllGather into ps_full[(L+1)%2], so mid-layer quarter
    # collectives never overwrite data still being gathered.
    ps_full = [
        nc.dram_tensor(f"ps_full{i}", [NPAD, D], F16, addr_space="Shared")
        for i in range(2)
    ]

    QROT = [0]
    ranges = [list(range(r, min(r + RP, NPAIR))) for r in range(0, NPAIR, RP)]

    with tile.TileContext(nc) as tc:
        with (
            tc.tile_pool(name="const", bufs=1) as cpool,
            tc.tile_pool(name="msg", bufs=3) as mpool,
            tc.tile_pool(name="idxp", bufs=3) as ipool,
            tc.tile_pool(name="sstr", bufs=3) as spool,
            tc.tile_pool(name="work", bufs=3) as wpool,
            tc.tile_pool(name="outp", bufs=4) as opool,
            tc.tile_pool(name="pz", bufs=2, space="PSUM") as pzpool,
            tc.tile_pool(name="ph", bufs=2, space="PSUM") as phpool,
            tc.tile_pool(name="pt", bufs=2, space="PSUM") as ptpool,
        ):
            # ---- constants ----
            ident_sb = cpool.tile([128, 128], F16)
            nc.sync.dma_start(out=ident_sb[:], in_=ident_ext[:, :])
            identf_sb = cpool.tile([128, 128], F32)
            nc.sync.dma_start(out=identf_sb[:], in_=identf_ext[:, :])
            w_sb = []
            for l in range(3):
                wt = cpool.tile([D, D], F16, tag=f"w{l}")
                nc.sync.dma_start(out=wt[:], in_=w_ext[l][:, :])
                w_sb.append(wt)
            b_sb = []
            for l in range(3):
                bt = cpool.tile([128, 1], F32, tag=f"b{l}")
                nc.sync.dma_start(out=bt[:], in_=b_ext[l][:, :])
                b_sb.append(bt)
            dinv_w = cpool.tile([128, NW], F32)
            nc.sync.dma_start(out=dinv_w[:], in_=dinvw_ext[:, :])

            # dinv_bc[p, d] = dinv[d] (host-expanded, resident)
            dinv_bc = cpool.tile([128, NB], F16)
            nc.sync.dma_start(out=dinv_bc[:], in_=dinvbc_ext[:, :])

            # ---- resident local ps block [128, NW*128] f16 ----
            ps_sb = cpool.tile([128, NW, 128], F16)

            # prologue: x16 is host-prescaled (dinv*x, fp16). Copy it to the
            # internal ps_loc (collectives cannot read IO tensors), AllGather,
            # and load the resident SBUF copy.
            nc.sync.dma_start(out=ps_loc[:, :], in_=x_ext[:, :])
            nc.gpsimd.collective_compute(
                "AllGather", mybir.AluOpType.bypass,
                replica_groups=[list(range(NCORES))],
                ins=[ps_loc.ap().opt()], outs=[ps_full[0].ap().opt()],
            )
            for w in range(NW):
                nc.sync.dma_start(
                    out=ps_sb[:, w, :], in_=x_ext[w * 128 : w * 128 + 128, :]
                )

            # ---- layers ----
            for layer in range(3):
                for rng_pairs in ranges:
                    ng_r = int(sum(gcell[p, k] for p in rng_pairs
                                   for k in range(nchunks)))
                    g0_s = int(cell_goff[rng_pairs[0], 0])
                    # S slice for this range [128, ng_r*PAIRW]
                    st_sb = spool.tile([128, ng_r * PAIRW], S_DTYPE, tag="s")
                    nc.sync.dma_start(
                        out=st_sb[:],
                        in_=s_ext[:, g0_s * PAIRW : (g0_s + ng_r) * PAIRW],
                    )
                    # gathers per chunk
                    mtiles = {}
                    for k in range(nchunks):
                        g_rk = int(sum(gcell[p, k] for p in rng_pairs))
                        if g_rk == 0:
                            continue
                        gk0 = int(chunk_goff[rng_pairs[0], k])
                        ni = g_rk * 128
                        it = ipool.tile([128, ni // 16], I16, tag=f"i{k}")
                        nc.sync.dma_start(
                            out=it[:],
                            in_=idx_ext[k][:, gk0 * 8 : gk0 * 8 + ni // 16],
                        )
                        mt = mpool.tile([128, g_rk, 128], F16, tag=f"m{k}")
                        c0 = k * CHUNK_ROWS
                        nc.gpsimd.dma_gather(
                            mt[:],
                            ps_full[layer % 2][c0 : c0 + CHUNK_ROWS, :],
                            it[:], ni, ni, D,
                            single_packet=False,
                            queue_num=QROT[0] % NQ,
                        )
                        QROT[0] += 1
                        mtiles[k] = (mt, gk0)

                    for p in rng_pairs:
                        zps = pzpool.tile([128, PAIRW], F32, tag="zacc")
                        ng = int(sum(gcell[p, k] for k in range(nchunks)))
                        gi = 0
                        for k in range(nchunks):
                            for j in range(int(gcell[p, k])):
                                mt, gk0 = mtiles[k]
                                slot = int(chunk_goff[p, k]) - gk0 + j
                                gcol = int(cell_goff[p, k]) + j - g0_s
                                nc.tensor.matmul(
                                    zps[:], mt[:, slot, :],
                                    st_sb[:, gcol * PAIRW
                                          : (gcol + 1) * PAIRW],
                                    start=(gi == 0), stop=False,
                                )
                                gi += 1
                        # self term: zps[:, h*128:...] += ps_w^T
                        for h in range(2):
                            w = p * 2 + h
                            if w >= NW:
                                break
                            nc.tensor.matmul(
                                zps[:, h * 128 : h * 128 + 128],
                                ps_sb[:, w, :], ident_sb[:],
                                start=(gi == 0), stop=(h == 1 or w == NW - 1),
                                skip_group_check=True,
                            )
                        # zsT = dinv_dst * zps  -> SBUF f16 (rhs of W matmul)
                        zsT = wpool.tile([128, PAIRW], F16, tag="zst")
                        c0 = p * PAIRW
                        nc.vector.tensor_mul(
                            zsT[:], zps[:], dinv_bc[:, c0 : c0 + PAIRW]
                        )
                        hps = phpool.tile([128, PAIRW], F32, tag="h")
                        nc.tensor.matmul(
                            hps[:], w_sb[layer][:], zsT[:],
                            start=True, stop=True,
                        )
                        hT = wpool.tile([128, PAIRW], F32, tag="ht")
                        if layer < 2:
                            nc.scalar.activation(
                                hT[:], hps[:],
                                mybir.ActivationFunctionType.Relu,
                                bias=b_sb[layer][:],
                            )
                        else:
                            nc.scalar.activation(
                                hT[:], hps[:],
                                mybir.ActivationFunctionType.Identity,
                                bias=b_sb[layer][:],
                            )
                        for h in range(2):
                            w = p * 2 + h
                            if w >= NW:
                                break
                            tp = ptpool.tile([128, 128], F32, tag="tp")
                            nc.tensor.transpose(
                                tp[:], hT[:, h * 128 : h * 128 + 128],
                                identf_sb[:],
                            )
                            if layer < 2:
                                nc.vector.tensor_scalar(
                                    ps_sb[:, w, :], tp[:],
                                    dinv_w[:, w : w + 1], None,
                                    op0=mybir.AluOpType.mult,
                                )
                                nc.sync.dma_start(
                                    out=ps_loc[w * 128 : w * 128 + 128, :],
                                    in_=ps_sb[:, w, :],
                                )
                            else:
                                ot = opool.tile([128, 128], F32, tag="oout")
                                nc.scalar.copy(out=ot[:], in_=tp[:])
                                nc.sync.dma_start(
                                    out=out_ext[w * 128 : w * 128 + 128, :],
                                    in_=ot[:],
                                )
                if layer < 2:
                    nc.gpsimd.collective_compute(
                        "AllGather", mybir.AluOpType.bypass,
                        replica_groups=[list(range(NCORES))],
                        ins=[ps_loc.ap().opt()],
                        outs=[ps_full[(layer + 1) % 2].ap().opt()],
                    )

    nc.finalize()
    return nc


_CACHE = {}
TRACE = False
LAST_EXEC_NS = None


def kernel(x, edge_index, W1, b1, W2, b2, W3, b3):
    global LAST_EXEC_NS
    x = np.asarray(x, dtype=np.float32)
    edge_index = np.asarray(edge_index)
    n_nodes = x.shape[0]

    ck = (n_nodes, edge_index.shape[1], hash(edge_index.tobytes()))
    if ck in _CACHE:
        meta, nc = _CACHE[ck]
    else:
        meta = _preprocess(edge_index, n_nodes)
        nc = _build(meta)
        _CACHE[ck] = (meta, nc)

    NB, NW, nb_real = meta["NB"], meta["NW"], meta["nb_real"]
    nchunks = meta["nchunks"]

    ident = np.eye(128)
    ws = [np.asarray(W1, np.float16), np.asarray(W2, np.float16),
          np.asarray(W3, np.float16)]
    bs = [np.asarray(b1, np.float32), np.asarray(b2, np.float32),
          np.asarray(b3, np.float32)]

    s_cast = meta["s_blob"].astype(mybir.dt.np(S_DTYPE))
    ident16 = ident.astype(np.float16)

    in_maps = []
    for c in range(NCORES):
        lo = c * nb_real
        hi = min(n_nodes, (c + 1) * nb_real)
        xb = np.zeros((NB, D), dtype=np.float16)
        dv = meta["dinv_bc"][c][0, : hi - lo, None]
        xb[: hi - lo] = (dv * x[lo:hi]).astype(np.float16)
        im = {
            "x16": xb,
            "dinvw": meta["dinv_w"][c],
            "dinvbc": np.ascontiguousarray(
                np.broadcast_to(meta["dinv_bc"][c], (128, meta["NB"]))
            ).astype(np.float16),
            "ident": ident16,
            "identf": ident.astype(np.float32),
            "sblob": s_cast[c],
        }
        for l in range(3):
            im[f"w{l}"] = ws[l]
            im[f"b{l}"] = bs[l].reshape(128, 1)
        for k in range(nchunks):
            im[f"idx{k}"] = meta["idx_wrapped"][k][c]
        in_maps.append(im)

    res = bass_utils.run_bass_kernel_spmd(
        nc, in_maps, core_ids=list(range(NCORES)), trace=TRACE
    )
    LAST_EXEC_NS = res.exec_time_ns

    out = np.empty((n_nodes, D), dtype=np.float32)
    for c in range(NCORES):
        lo = c * nb_real
        hi = min(n_nodes, (c + 1) * nb_real)
        out[lo:hi] = res.results[c]["out"][: hi - lo]
    return out
